# revision 1
# baseline (speedup 1.0000x reference)
"""Trainium2 Bass kernel for nn_Net_14869176779172 (moe_routing).

Computes, for x[B=1024, D=4096, S=60], W[D, S], soma_w[D], soma_b[1]:
    d[b, j]  = sum_s x[b, j, s] * W[j, s]          (per-dendrite dot)
    r        = relu(d)
    act[b,j] = sigmoid(r)        for j < 1638      (first 40% of dendrites)
             = sqrt(r)           otherwise
    out[b]   = act[b, :] @ soma_w + soma_b         -> [B, 1]

Sharding: pure data-parallel over batch across 8 NeuronCores (128 batch
rows per core); W / soma_w / soma_b replicated.

Per-core layout: batch on the 128 SBUF partitions (so all x DMAs are
fully contiguous per partition).  W is broadcast across partitions with
a ones[1,128] matmul on the (otherwise idle) TensorEngine into PSUM;
the VectorEngine does the x*W multiply (reading W from PSUM) and the
segmented reduction over S.  ScalarEngine applies sigmoid/sqrt.
"""

import numpy as np

import concourse.bacc as bacc
import concourse.bass as bass
import concourse.tile as tile
from concourse import mybir
from concourse.bass_utils import run_bass_kernel_spmd

# Problem constants (hardcoded per harness contract).
B_TOTAL = 1024
N_CORES = 8
B = B_TOTAL // N_CORES  # 128 batch rows per core
D = 4096
S = 60
CUT = int(D * 0.4)  # 1638: first CUT dendrites use sigmoid, rest sqrt

P = 128  # SBUF partitions

# Tiling: a "piece" is 32 dendrites (32*60 = 1920 floats per partition),
# whose broadcast weights fit in 4 PSUM banks ([128, 4, 512] fp32).
PIECE_D = 32
PIECE_F = PIECE_D * S  # 1920
N_PIECES = D // PIECE_D  # 128
# x is DMA'd in chunks of 4 pieces (128 dendrites, 3.9 MB per DMA).
CHUNK_PIECES = 4
CHUNK_F = CHUNK_PIECES * PIECE_F  # 7680
CHUNK_D = CHUNK_PIECES * PIECE_D  # 128
N_CHUNKS = D // CHUNK_D  # 32

FP32 = mybir.dt.float32
FP16 = mybir.dt.float16


def _build_program():
    nc = bacc.Bacc(
        "TRN2",
        target_bir_lowering=False,
        debug=False,
        enable_asserts=False,
        num_devices=N_CORES,
    )

    x_d = nc.dram_tensor("x", [B, D, S], FP32, kind="ExternalInput")
    w_d = nc.dram_tensor("W16", [D, S], FP16, kind="ExternalInput")
    sw_d = nc.dram_tensor("soma_w", [D], FP32, kind="ExternalInput")
    sb_d = nc.dram_tensor("soma_b", [1], FP32, kind="ExternalInput")
    oh_d = nc.dram_tensor("onehot", [32, 32 * P], FP16, kind="ExternalInput")
    out_d = nc.dram_tensor("out", [B, 1], FP32, kind="ExternalOutput")

    x_flat = x_d.ap().rearrange("b d s -> b (d s)")  # [128, 245760]
    w_flat = w_d.ap().rearrange("d s -> (d s)")  # [245760]

    with tile.TileContext(nc) as tc:
        with (
            tc.tile_pool(name="singles", bufs=1) as singles,
            tc.tile_pool(name="xpool", bufs=4) as xpool,
            tc.tile_pool(name="ypool", bufs=2) as ypool,
            tc.tile_pool(name="y2pool", bufs=2) as y2pool,
            tc.tile_pool(name="yspool", bufs=3) as yspool,
            tc.tile_pool(name="w16pool", bufs=2) as w16pool,
            tc.tile_pool(name="wpsum", bufs=2, space="PSUM") as wpsum,
        ):
            # ---- constants / small inputs ----
            # One-hot row-selector stationaries: onehot_t[:, r*128:(r+1)*128]
            # is a [32, 128] matrix whose row r is all-ones.  Matmul with it
            # broadcasts partition r of the rhs to all 128 output partitions.
            # (Supplied from the host: engines can't memset at partition r>0.)
            onehot_t = singles.tile([32, 32 * P], FP16)
            nc.sync.dma_start(out=onehot_t, in_=oh_d.ap())

            # W, reshaped [32, 7680]: partition p holds dendrite pieces
            # 4p..4p+3 (each piece = 32 dendrites * 60 syn = 1920 floats).
            w_sb = singles.tile([32, 4 * PIECE_F], FP16)
            nc.sync.dma_start(
                out=w_sb, in_=w_flat.rearrange("(p f) -> p f", p=32)
            )

            # soma_w broadcast to all partitions: [128, 4096] (2 MB DMA).
            swb = singles.tile([P, D], FP32)
            sw_ap = sw_d.ap()
            nc.sync.dma_start(
                out=swb,
                in_=bass.AP(
                    tensor=sw_ap.tensor, offset=sw_ap.offset, ap=[[0, P], *sw_ap.ap]
                ),
            )
            # soma_b broadcast: [128, 1]
            sbb = singles.tile([P, 1], FP32)
            sb_ap = sb_d.ap()
            nc.sync.dma_start(
                out=sbb,
                in_=bass.AP(
                    tensor=sb_ap.tensor, offset=sb_ap.offset, ap=[[0, P], *sb_ap.ap]
                ),
            )

            # accumulator for all dendrite outputs of this core's batch rows
            z_buf = singles.tile([P, D], FP32)
            # running partial sums for the soma dot product
            acc = singles.tile([P, P], FP32)
            nc.vector.memset(acc, 0.0)

            # ---- main loop: multiply + segmented reduce ----
            for c in range(N_CHUNKS):
                # x chunk, cast fp32 -> fp16 during the DMA (SWDGE)
                xc = xpool.tile([P, CHUNK_F], FP16)
                nc.gpsimd.dma_start(
                    out=xc, in_=x_flat[:, c * CHUNK_F : (c + 1) * CHUNK_F]
                )
                # Broadcast this chunk's W across partitions via TensorE,
                # evacuating PSUM -> SBUF (fp16) on ScalarE piece by piece.
                w16 = w16pool.tile([P, CHUNK_F], FP16)
                for k in range(CHUNK_PIECES):
                    pi = c * CHUNK_PIECES + k
                    p_row, g = pi // 4, pi % 4
                    wb = wpsum.tile([P, 4, 512], FP32)
                    for j in range(4):
                        nc.tensor.matmul(
                            wb[:, j, 0:480],
                            onehot_t[:, p_row * P : (p_row + 1) * P],
                            w_sb[:, g * PIECE_F + j * 480 : g * PIECE_F + (j + 1) * 480],
                        )
                    nc.scalar.copy(
                        out=w16[:, k * PIECE_F : (k + 1) * PIECE_F].rearrange(
                            "p (a f) -> p a f", a=4
                        ),
                        in_=wb[:, :, 0:480],
                    )
                # one 2x-mode multiply for the whole chunk
                y = ypool.tile([P, CHUNK_F], FP16)
                nc.vector.tensor_mul(y, xc, w16)
                # first reduction step y2[d,s] = y[d,s] + y[d,s+30]; alternate
                # chunks go to GpSimd to offload the VectorE
                y3 = y.rearrange("p (d s) -> p d s", s=S)
                y2 = y2pool.tile([P, CHUNK_D * (S // 2)], FP16)
                half_eng = nc.gpsimd if (c % 2 == 0) else nc.vector
                half_eng.tensor_add(
                    y2.rearrange("p (d s) -> p d s", s=S // 2),
                    y3[:, :, 0 : S // 2],
                    y3[:, :, S // 2 : S],
                )
                nc.vector.tensor_reduce(
                    out=z_buf[:, c * CHUNK_D : (c + 1) * CHUNK_D],
                    in_=y2.rearrange("p (d s) -> p d s", s=S // 2),
                    axis=mybir.AxisListType.X,
                    op=mybir.AluOpType.add,
                )

                # ---- per-chunk activations + soma partial (overlap the tail)
                d0, d1 = c * CHUNK_D, (c + 1) * CHUNK_D
                zc = z_buf[:, d0:d1]
                nc.vector.tensor_scalar_max(out=zc, in0=zc, scalar1=0.0)
                # sigmoid region [0, CUT), sqrt region [CUT, D)
                if d1 <= CUT:
                    nc.scalar.activation(
                        out=zc, in_=zc, func=mybir.ActivationFunctionType.Sigmoid
                    )
                elif d0 >= CUT:
                    nc.scalar.activation(
                        out=zc, in_=zc, func=mybir.ActivationFunctionType.Sqrt
                    )
                else:
                    nc.scalar.activation(
                        out=z_buf[:, d0:CUT],
                        in_=z_buf[:, d0:CUT],
                        func=mybir.ActivationFunctionType.Sigmoid,
                    )
                    nc.scalar.activation(
                        out=z_buf[:, CUT:d1],
                        in_=z_buf[:, CUT:d1],
                        func=mybir.ActivationFunctionType.Sqrt,
                    )
                # acc[:, j] += sum over this chunk's dendrite groups of act*soma_w
                ysc = yspool.tile([P, CHUNK_D], FP32)
                nc.vector.tensor_mul(ysc, zc, swb[:, d0:d1])
                for g in range(CHUNK_D // P):
                    nc.vector.tensor_add(acc, acc, ysc[:, g * P : (g + 1) * P])

            # ---- soma: out = sum(acc) + soma_b ----
            zsum = singles.tile([P, 1], FP32)
            nc.vector.tensor_reduce(
                out=zsum,
                in_=acc,
                axis=mybir.AxisListType.X,
                op=mybir.AluOpType.add,
            )
            out_sb = singles.tile([P, 1], FP32)
            nc.vector.tensor_add(out_sb, zsum, sbb)
            nc.sync.dma_start(out=out_d.ap().rearrange("b one -> b one"), in_=out_sb)

    nc.compile()
    return nc


_NC_CACHE = None


def _get_program():
    global _NC_CACHE
    if _NC_CACHE is None:
        _NC_CACHE = _build_program()
    return _NC_CACHE


def kernel(x, W, soma_w, soma_b, _trace=False):
    nc = _get_program()
    x = np.ascontiguousarray(x, dtype=np.float32)
    W = np.ascontiguousarray(W, dtype=np.float32)
    soma_w = np.ascontiguousarray(soma_w, dtype=np.float32)
    soma_b = np.ascontiguousarray(soma_b, dtype=np.float32)

    onehot = np.ascontiguousarray(
        np.repeat(np.eye(32, dtype=np.float16), P, axis=1)
    )  # [32, 32*128]
    in_maps = [
        {
            "x": np.ascontiguousarray(x[i * B : (i + 1) * B]),
            "W16": W.astype(np.float16),
            "soma_w": soma_w,
            "soma_b": soma_b,
            "onehot": onehot,
        }
        for i in range(N_CORES)
    ]
    res = run_bass_kernel_spmd(
        nc, in_maps, core_ids=list(range(N_CORES)), trace=_trace
    )
    out = np.concatenate([r["out"] for r in res.results], axis=0)
    if _trace:
        kernel.last_results = res
    return out.astype(np.float32)

